# revision 24
# baseline (speedup 1.0000x reference)
"""Multi-head attention (B=2, S=2048, D=1024, H=16) on 8 Trainium2 NeuronCores.

Sharding: tensor-parallel on heads (4 groups of 4 heads) x data-parallel on
batch (2) -> 8 cores. Each core computes QKV projections for its head slice,
attention for its 4 heads, and a partial output projection; the host sums the
4 partials per batch element (the tensor-parallel allreduce) and adds bo.

All matmul operands are fp16 (fp32 PSUM accumulation). Scores are computed
transposed (ST[k,q] = KT_h.T @ QT_h) so softmax exp feeds attn@V directly as
the stationary operand with no transposes; a ones-column appended to V makes
the same matmul accumulate the softmax denominators.
"""

import numpy as np

import concourse.bass as bass  # noqa: F401
import concourse.tile as tile
from concourse import bacc, mybir
from concourse.bass_utils import run_bass_kernel_spmd
D_MODEL = 1024
NUM_HEADS = 16
DK = 64
B, S = 2, 2048
N_CORES = 8
GROUPS = 4                 # head groups (tensor parallel)
GW = D_MODEL // GROUPS     # 256 features per group = 4 heads
HPG = GROUPS               # heads per group = 4

F32 = mybir.dt.float32
BF16 = mybir.dt.float16  # 16-bit matmul operand dtype
EXPF = mybir.ActivationFunctionType.Exp
MULT = mybir.AluOpType.mult
ADD = mybir.AluOpType.add


def _emit(nc, tc, ctx):
    P = 128
    xqT = nc.dram_tensor("xqT", [D_MODEL, S], BF16, kind="ExternalInput")
    xkT = nc.dram_tensor("xkT", [D_MODEL, S], BF16, kind="ExternalInput")
    xvT = nc.dram_tensor("xvT", [D_MODEL, S], BF16, kind="ExternalInput")
    wqT = nc.dram_tensor("wqT", [D_MODEL, GW], BF16, kind="ExternalInput")
    wkT = nc.dram_tensor("wkT", [D_MODEL, GW], BF16, kind="ExternalInput")
    wvT = nc.dram_tensor("wvT", [D_MODEL, GW], BF16, kind="ExternalInput")
    woT = nc.dram_tensor("woT", [GW, D_MODEL], BF16, kind="ExternalInput")
    bq2 = nc.dram_tensor("bq2", [P, 2], F32, kind="ExternalInput")
    bk2 = nc.dram_tensor("bk2", [P, 2], F32, kind="ExternalInput")
    bvr = nc.dram_tensor("bvr", [1, GW], F32, kind="ExternalInput")
    out = nc.dram_tensor("out", [S, D_MODEL], F32, kind="ExternalOutput")

    consts = ctx.enter_context(tc.tile_pool(name="consts", bufs=1))
    persist = ctx.enter_context(tc.tile_pool(name="persist", bufs=1))
    xs = ctx.enter_context(tc.tile_pool(name="xs", bufs=5))
    sx = ctx.enter_context(tc.tile_pool(name="stexp", bufs=3))
    nrm = ctx.enter_context(tc.tile_pool(name="nrm", bufs=2))
    outp = ctx.enter_context(tc.tile_pool(name="outp", bufs=3))
    psA = ctx.enter_context(tc.tile_pool(name="psA", bufs=2, space="PSUM"))
    psB = ctx.enter_context(tc.tile_pool(name="psB", bufs=2, space="PSUM"))

    # ---- constants / weights -------------------------------------------
    wq_sb = consts.tile([P, 8, GW], BF16)
    wk_sb = consts.tile([P, 8, GW], BF16)
    wv_sb = consts.tile([P, 8, GW], BF16)
    wo_sb = consts.tile([P, 2, D_MODEL], BF16)
    nc.sync.dma_start(wq_sb[:], wqT[:].rearrange("(c p) j -> p c j", p=P))
    bq_sb = consts.tile([P, 2], F32)
    bk_sb = consts.tile([P, 2], F32)
    nc.sync.dma_start(bq_sb[:], bq2[:])
    nc.sync.dma_start(bk_sb[:], bk2[:])
    bv_row = consts.tile([1, GW], F32)
    nc.sync.dma_start(bv_row[:], bvr[:])
    bvb = consts.tile([P, GW], F32)
    nc.gpsimd.partition_broadcast(bvb[:], bv_row[:])

    # persistent activations (QT doubles as O.T after attention), split by
    # feature chunk / s-half so attention can start before phase 1 finishes
    QTs = [persist.tile([P, S], BF16, name=f"QT{j}") for j in range(2)]
    # KT split by (feature chunk, s-half): scores over k<1024 need only K(sb0)
    KT4 = [[persist.tile([P, 1024], BF16, name=f"KT{j}_{hh}") for hh in range(2)]
           for j in range(2)]
    Vaugs = [persist.tile([P, 8, HPG, DK + 1], BF16, name=f"Vaug{v}")
             for v in range(2)]
    ones_f32 = consts.tile([P, 8, HPG], F32)
    nc.vector.memset(ones_f32[:], 1.0)
    for v in range(2):
        nc.vector.tensor_scalar_add(Vaugs[v][:, :, :, DK], ones_f32[:], 0.0)

    xqT_r = xqT[:].rearrange("(c p) s -> p c s", p=P)
    xkT_r = xkT[:].rearrange("(c p) s -> p c s", p=P)
    xvT_r = xvT[:].rearrange("(c p) s -> p c s", p=P)

    # ---- phase 1: QKV projections (Q fully, then K, then V) -------------
    def proj_block(name, x_r, w_sb, b_sb, dstTs, sb):
        ssl = slice(sb * 1024, (sb + 1) * 1024)
        ps = [psA.tile([P, 1024], F32, tag="psA", name=f"ps_{name}{sb}{j}")
              for j in range(2)]
        for i in range(8):
            xt = xs.tile([P, 1024], BF16, tag="xs")
            nc.sync.dma_start(xt[:], x_r[:, i, ssl])
            for j in range(2):
                for ns in range(2):
                    nc.tensor.matmul(
                        ps[j][:, ns * 512:(ns + 1) * 512],
                        w_sb[:, i, j * P:(j + 1) * P],
                        xt[:, ns * 512:(ns + 1) * 512],
                        start=(i == 0), stop=(i == 7),
                    )
        for j in range(2):
            if name == "k":
                nc.vector.tensor_scalar_add(dstTs[j][sb][:, :], ps[j][:],
                                            b_sb[:, j:j + 1])
            else:
                nc.vector.tensor_scalar_add(dstTs[j][:, ssl], ps[j][:],
                                            b_sb[:, j:j + 1])

    nc.sync.dma_start(wk_sb[:], wkT[:].rearrange("(c p) j -> p c j", p=P))
    proj_block("q", xqT_r, wq_sb, bq_sb, QTs, 0)
    nc.sync.dma_start(wv_sb[:], wvT[:].rearrange("(c p) j -> p c j", p=P))
    proj_block("k", xkT_r, wk_sb, bk_sb, KT4, 0)
    # V: natural layout, s on partitions; emitted before K(sb1) so V's
    # matmuls run while xk(sb1) streams in, and attnV never waits on V
    for g_ss in range(16):
        xvt = xs.tile([P, 8, P], BF16, tag="xs", name=f"xv{g_ss}")
        nc.sync.dma_start(xvt[:], xvT_r[:, :, g_ss * P:(g_ss + 1) * P])
        pv = psB.tile([P, 1024], F32, tag="psB")
        for i in range(8):
            nc.tensor.matmul(
                pv[:, 0:GW], xvt[:, i, :], wv_sb[:, i, :],
                start=(i == 0), stop=(i == 7),
            )
        nc.vector.tensor_tensor(
            Vaugs[g_ss // 8][:, g_ss % 8, :, 0:DK],
            pv[:, 0:GW].rearrange("p (h d) -> p h d", h=HPG),
            bvb[:].rearrange("p (h d) -> p h d", h=HPG),
            ADD,
        )
    proj_block("k", xkT_r, wk_sb, bk_sb, KT4, 1)
    nc.sync.dma_start(wo_sb[:], woT[:].rearrange("(c p) m -> p c m", p=P))

    # ---- phase 2: attention per (head, q-block) ------------------------
    def attn_qblock(qb, defer=False):
        for h in range(HPG):
            pr = 64 * (h % 2)   # partition offset of this head's features
            jc = h // 2         # feature chunk
            qsl = slice(qb * 1024, (qb + 1) * 1024)
            st = sx.tile([P, 16, 1024], BF16, tag="stexp")
            po = psB.tile([P, 1024], F32, tag="psB")
            for k in range(16):
                pst = psA.tile([P, 1024], F32, tag="psA")
                for ns in range(2):
                    nc.tensor.matmul(
                        pst[:, ns * 512:(ns + 1) * 512],
                        KT4[jc][k // 8][pr:pr + DK, (k % 8) * P:(k % 8 + 1) * P],
                        QTs[jc][pr:pr + DK, qb * 1024 + ns * 512:
                                qb * 1024 + (ns + 1) * 512],
                        start=True, stop=True,
                    )
                nc.scalar.activation(out=st[:, k, :], in_=pst[:], func=EXPF,
                                     scale=0.125)
                for ns in range(2):
                    nc.tensor.matmul(
                        po[0:DK + 1, ns * 512:(ns + 1) * 512],
                        Vaugs[k // 8][:, k % 8, h, :],
                        st[:, k, ns * 512:(ns + 1) * 512],
                        start=(k == 0), stop=(k == 15),
                    )
            # normalize: row DK of po holds softmax denominators
            bc = nrm.tile([DK, 1024], F32, tag="bcast")
            dn = nrm.tile([1, 1024], F32, tag="denom")
            nc.vector.tensor_copy(out=dn[:], in_=po[DK:DK + 1, :])
            nc.vector.reciprocal_approx_fast(bc[0:1, :], dn[:])
            nc.gpsimd.partition_broadcast(bc[:], bc[0:1, :])
            # write O.T for this (head, q-block) into QT's now-dead region
            nc.vector.tensor_tensor(QTs[jc][pr:pr + DK, qsl], po[0:DK, :], bc[:],
                                    MULT)

    def oproj_qblock(qb):
        # output projection for one q-block (overlaps other work)
        for sc in range(qb * 8, (qb + 1) * 8):
            pso = psB.tile([P, 1024], F32, tag="psB")
            for hd in range(2):
                for ms in range(2):
                    nc.tensor.matmul(
                        pso[:, ms * 512:(ms + 1) * 512],
                        QTs[hd][:, sc * P:(sc + 1) * P],
                        wo_sb[:, hd, ms * 512:(ms + 1) * 512],
                        start=(hd == 0), stop=(hd == 1),
                    )
            ot = outp.tile([P, 1024], F32, tag="osb")
            if qb == 1 and sc % 2 == 1:
                nc.scalar.copy(out=ot[:], in_=pso[:])
            else:
                nc.vector.tensor_copy(out=ot[:], in_=pso[:])
            nc.sync.dma_start(out[sc * P:(sc + 1) * P, :], ot[:])

    attn_qblock(0)
    # Q projection for the second s-half, hidden under qb0 attention; O-proj
    # for qb0 comes after it so qb1's first scores aren't serialized behind it
    proj_block("q", xqT_r, wq_sb, bq_sb, QTs, 1)
    oproj_qblock(0)
    attn_qblock(1)
    oproj_qblock(1)



_prog_cache = {}


def _build_program():
    if "nc" not in _prog_cache:
        from contextlib import ExitStack
        nc = bacc.Bacc("TRN2", target_bir_lowering=False)
        with tile.TileContext(nc) as tc:
            with ExitStack() as ctx:
                _emit(nc, tc, ctx)
        nc.compile()
        _prog_cache["nc"] = nc
    return _prog_cache["nc"]


def make_in_maps(query, key, value, Wq, bq, Wk, bk, Wv, bv, Wo, bo):
    query, key, value = (np.asarray(t, np.float32) for t in (query, key, value))
    Wq, Wk, Wv, Wo = (np.asarray(t, np.float32) for t in (Wq, Wk, Wv, Wo))
    bq, bk, bv = (np.asarray(t, np.float32) for t in (bq, bk, bv))
    xT = {b: {} for b in range(B)}
    for b in range(B):
        xT[b]["q"] = np.ascontiguousarray(query[b].T).astype(np.float16)
        xT[b]["k"] = np.ascontiguousarray(key[b].T).astype(np.float16)
        xT[b]["v"] = np.ascontiguousarray(value[b].T).astype(np.float16)
    in_maps = []
    for c in range(N_CORES):
        b, g = divmod(c, GROUPS)
        gs = slice(g * GW, (g + 1) * GW)
        in_maps.append({
            "xqT": xT[b]["q"], "xkT": xT[b]["k"], "xvT": xT[b]["v"],
            "wqT": np.ascontiguousarray(Wq[gs, :].T).astype(np.float16),
            "wkT": np.ascontiguousarray(Wk[gs, :].T).astype(np.float16),
            "wvT": np.ascontiguousarray(Wv[gs, :].T).astype(np.float16),
            "woT": np.ascontiguousarray(Wo[:, gs].T).astype(np.float16),
            "bq2": np.ascontiguousarray(bq[gs].reshape(2, 128).T),
            "bk2": np.ascontiguousarray(bk[gs].reshape(2, 128).T),
            "bvr": np.ascontiguousarray(bv[gs].reshape(1, GW)),
        })
    return in_maps


def run_on_hw(in_maps, trace=False, **kw):
    nc = _build_program()
    return run_bass_kernel_spmd(nc, in_maps, core_ids=list(range(N_CORES)),
                                trace=trace, **kw)


def kernel(query, key, value, Wq, bq, Wk, bk, Wv, bv, Wo, bo):
    in_maps = make_in_maps(query, key, value, Wq, bq, Wk, bk, Wv, bv, Wo, bo)
    res = run_on_hw(in_maps)
    out = np.zeros((B, S, D_MODEL), np.float32)
    for c in range(N_CORES):
        out[c // GROUPS] += res.results[c]["out"]
    out += np.asarray(bo, np.float32)
    return out


if __name__ == "__main__":
    # self-check against a pure-numpy reference
    rng = np.random.default_rng(0)
    sc = 1.0 / np.sqrt(D_MODEL)
    inp = dict(
        query=rng.standard_normal((B, S, D_MODEL), np.float32),
        key=rng.standard_normal((B, S, D_MODEL), np.float32),
        value=rng.standard_normal((B, S, D_MODEL), np.float32),
        Wq=(rng.standard_normal((D_MODEL, D_MODEL)) * sc).astype(np.float32),
        bq=rng.standard_normal(D_MODEL).astype(np.float32) * 0.1,
        Wk=(rng.standard_normal((D_MODEL, D_MODEL)) * sc).astype(np.float32),
        bk=rng.standard_normal(D_MODEL).astype(np.float32) * 0.1,
        Wv=(rng.standard_normal((D_MODEL, D_MODEL)) * sc).astype(np.float32),
        bv=rng.standard_normal(D_MODEL).astype(np.float32) * 0.1,
        Wo=(rng.standard_normal((D_MODEL, D_MODEL)) * sc).astype(np.float32),
        bo=rng.standard_normal(D_MODEL).astype(np.float32) * 0.1,
    )

    def np_ref(query, key, value, Wq, bq, Wk, bk, Wv, bv, Wo, bo):
        q = query.astype(np.float64) @ Wq.T.astype(np.float64) + bq
        k = key.astype(np.float64) @ Wk.T.astype(np.float64) + bk
        v = value.astype(np.float64) @ Wv.T.astype(np.float64) + bv
        q = q.reshape(B, S, NUM_HEADS, DK).transpose(0, 2, 1, 3)
        k = k.reshape(B, S, NUM_HEADS, DK).transpose(0, 2, 1, 3)
        v = v.reshape(B, S, NUM_HEADS, DK).transpose(0, 2, 1, 3)
        sc_ = np.einsum("bhqd,bhkd->bhqk", q, k) / np.sqrt(DK)
        sc_ -= sc_.max(-1, keepdims=True)
        a = np.exp(sc_)
        a /= a.sum(-1, keepdims=True)
        o = np.einsum("bhqk,bhkd->bhqd", a, v)
        o = o.transpose(0, 2, 1, 3).reshape(B, S, D_MODEL)
        return o @ Wo.T.astype(np.float64) + bo

    exp = np_ref(**inp)
    got = kernel(**inp)
    scale = np.abs(exp).max()
    err = np.abs(got - exp)
    print(f"max abs err {err.max():.4e}  rel {err.max() / scale:.4e}  "
          f"mean rel {err.mean() / scale:.4e}")


# revision 25
# speedup vs baseline: 1.1511x; 1.1511x over previous
"""Multi-head attention (B=2, S=2048, D=1024, H=16) on 8 Trainium2 NeuronCores.

Sharding: tensor-parallel on heads (4 groups of 4 heads) x data-parallel on
batch (2) -> 8 cores. Each core computes QKV projections for its head slice,
attention for its 4 heads, and a partial output projection; the host sums the
4 partials per batch element (the tensor-parallel allreduce) and adds bo.

All matmul operands are fp16 (fp32 PSUM accumulation). Scores are computed
transposed (ST[k,q] = KT_h.T @ QT_h) so softmax exp feeds attn@V directly as
the stationary operand with no transposes; a ones-column appended to V makes
the same matmul accumulate the softmax denominators.
"""

import numpy as np

import concourse.bass as bass  # noqa: F401
import concourse.tile as tile
from concourse import bacc, mybir
from concourse.bass_utils import run_bass_kernel_spmd
D_MODEL = 1024
NUM_HEADS = 16
DK = 64
B, S = 2, 2048
N_CORES = 8
GROUPS = 4                 # head groups (tensor parallel)
GW = D_MODEL // GROUPS     # 256 features per group = 4 heads
HPG = GROUPS               # heads per group = 4

F32 = mybir.dt.float32
BF16 = mybir.dt.float16  # 16-bit matmul operand dtype
EXPF = mybir.ActivationFunctionType.Exp
MULT = mybir.AluOpType.mult
ADD = mybir.AluOpType.add


def _emit(nc, tc, ctx):
    P = 128
    xqT = nc.dram_tensor("xqT", [D_MODEL, S], BF16, kind="ExternalInput")
    xkT = nc.dram_tensor("xkT", [D_MODEL, S], BF16, kind="ExternalInput")
    xvT = nc.dram_tensor("xvT", [D_MODEL, S], BF16, kind="ExternalInput")
    wqT = nc.dram_tensor("wqT", [D_MODEL, GW], BF16, kind="ExternalInput")
    wkT = nc.dram_tensor("wkT", [D_MODEL, GW], BF16, kind="ExternalInput")
    wvT = nc.dram_tensor("wvT", [D_MODEL, GW], BF16, kind="ExternalInput")
    woT = nc.dram_tensor("woT", [GW, D_MODEL], BF16, kind="ExternalInput")
    bq2 = nc.dram_tensor("bq2", [P, 2], F32, kind="ExternalInput")
    bk2 = nc.dram_tensor("bk2", [P, 2], F32, kind="ExternalInput")
    bvr = nc.dram_tensor("bvr", [1, GW], F32, kind="ExternalInput")
    out = nc.dram_tensor("out", [S, D_MODEL], F32, kind="ExternalOutput")

    consts = ctx.enter_context(tc.tile_pool(name="consts", bufs=1))
    persist = ctx.enter_context(tc.tile_pool(name="persist", bufs=1))
    xs = ctx.enter_context(tc.tile_pool(name="xs", bufs=5))
    sx = ctx.enter_context(tc.tile_pool(name="stexp", bufs=3))
    nrm = ctx.enter_context(tc.tile_pool(name="nrm", bufs=2))
    outp = ctx.enter_context(tc.tile_pool(name="outp", bufs=3))
    psA = ctx.enter_context(tc.tile_pool(name="psA", bufs=2, space="PSUM"))
    psB = ctx.enter_context(tc.tile_pool(name="psB", bufs=2, space="PSUM"))

    # ---- constants / weights -------------------------------------------
    wq_sb = consts.tile([P, 8, GW], BF16)
    wk_sb = consts.tile([P, 8, GW], BF16)
    wv_sb = consts.tile([P, 8, GW], BF16)
    wo_sb = consts.tile([P, 2, D_MODEL], BF16)
    nc.sync.dma_start(wq_sb[:], wqT[:].rearrange("(c p) j -> p c j", p=P))
    bq_sb = consts.tile([P, 2], F32)
    bk_sb = consts.tile([P, 2], F32)
    nc.sync.dma_start(bq_sb[:], bq2[:])
    nc.sync.dma_start(bk_sb[:], bk2[:])
    bv_row = consts.tile([1, GW], F32)
    nc.sync.dma_start(bv_row[:], bvr[:])
    bvb = consts.tile([P, GW], F32)
    nc.gpsimd.partition_broadcast(bvb[:], bv_row[:])

    # persistent activations (QT doubles as O.T after attention), split by
    # feature chunk / s-half so attention can start before phase 1 finishes
    QTs = [persist.tile([P, S], BF16, name=f"QT{j}") for j in range(2)]
    # KT split by (feature chunk, s-half): scores over k<1024 need only K(sb0)
    KT4 = [[persist.tile([P, 1024], BF16, name=f"KT{j}_{hh}") for hh in range(2)]
           for j in range(2)]
    Vaugs = [persist.tile([P, 8, HPG, DK + 1], BF16, name=f"Vaug{v}")
             for v in range(2)]
    ones_f32 = consts.tile([P, 8, HPG], F32)
    nc.vector.memset(ones_f32[:], 1.0)
    for v in range(2):
        nc.vector.tensor_scalar_add(Vaugs[v][:, :, :, DK], ones_f32[:], 0.0)

    xqT_r = xqT[:].rearrange("(c p) s -> p c s", p=P)
    xkT_r = xkT[:].rearrange("(c p) s -> p c s", p=P)
    xvT_r = xvT[:].rearrange("(c p) s -> p c s", p=P)

    # ---- phase 1: QKV projections (Q fully, then K, then V) -------------
    def proj_block(name, x_r, w_sb, b_sb, dstTs, sb):
        ssl = slice(sb * 1024, (sb + 1) * 1024)
        ps = [psA.tile([P, 1024], F32, tag="psA", name=f"ps_{name}{sb}{j}")
              for j in range(2)]
        for i in range(8):
            xt = xs.tile([P, 1024], BF16, tag="xs")
            nc.sync.dma_start(xt[:], x_r[:, i, ssl])
            for j in range(2):
                for ns in range(2):
                    nc.tensor.matmul(
                        ps[j][:, ns * 512:(ns + 1) * 512],
                        w_sb[:, i, j * P:(j + 1) * P],
                        xt[:, ns * 512:(ns + 1) * 512],
                        start=(i == 0), stop=(i == 7),
                    )
        for j in range(2):
            if name == "k":
                nc.vector.tensor_scalar_add(dstTs[j][sb][:, :], ps[j][:],
                                            b_sb[:, j:j + 1])
            else:
                nc.vector.tensor_scalar_add(dstTs[j][:, ssl], ps[j][:],
                                            b_sb[:, j:j + 1])

    nc.sync.dma_start(wk_sb[:], wkT[:].rearrange("(c p) j -> p c j", p=P))
    proj_block("q", xqT_r, wq_sb, bq_sb, QTs, 0)
    nc.sync.dma_start(wv_sb[:], wvT[:].rearrange("(c p) j -> p c j", p=P))
    proj_block("k", xkT_r, wk_sb, bk_sb, KT4, 0)
    proj_block("k", xkT_r, wk_sb, bk_sb, KT4, 1)
    nc.sync.dma_start(wo_sb[:], woT[:].rearrange("(c p) m -> p c m", p=P))
    # V: natural layout, s on partitions
    for g_ss in range(16):
        xvt = xs.tile([P, 8, P], BF16, tag="xs", name=f"xv{g_ss}")
        nc.sync.dma_start(xvt[:], xvT_r[:, :, g_ss * P:(g_ss + 1) * P])
        pv = psB.tile([P, 1024], F32, tag="psB")
        for i in range(8):
            nc.tensor.matmul(
                pv[:, 0:GW], xvt[:, i, :], wv_sb[:, i, :],
                start=(i == 0), stop=(i == 7),
            )
        nc.vector.tensor_tensor(
            Vaugs[g_ss // 8][:, g_ss % 8, :, 0:DK],
            pv[:, 0:GW].rearrange("p (h d) -> p h d", h=HPG),
            bvb[:].rearrange("p (h d) -> p h d", h=HPG),
            ADD,
        )

    # ---- phase 2: attention per (head, q-block) ------------------------
    def attn_qblock(qb, defer=False):
        for h in range(HPG):
            pr = 64 * (h % 2)   # partition offset of this head's features
            jc = h // 2         # feature chunk
            qsl = slice(qb * 1024, (qb + 1) * 1024)
            st = sx.tile([P, 16, 1024], BF16, tag="stexp")
            po = psB.tile([P, 1024], F32, tag="psB")
            for k in range(16):
                pst = psA.tile([P, 1024], F32, tag="psA")
                for ns in range(2):
                    nc.tensor.matmul(
                        pst[:, ns * 512:(ns + 1) * 512],
                        KT4[jc][k // 8][pr:pr + DK, (k % 8) * P:(k % 8 + 1) * P],
                        QTs[jc][pr:pr + DK, qb * 1024 + ns * 512:
                                qb * 1024 + (ns + 1) * 512],
                        start=True, stop=True,
                    )
                nc.scalar.activation(out=st[:, k, :], in_=pst[:], func=EXPF,
                                     scale=0.125)
                for ns in range(2):
                    nc.tensor.matmul(
                        po[0:DK + 1, ns * 512:(ns + 1) * 512],
                        Vaugs[k // 8][:, k % 8, h, :],
                        st[:, k, ns * 512:(ns + 1) * 512],
                        start=(k == 0), stop=(k == 15),
                    )
            # normalize: row DK of po holds softmax denominators
            bc = nrm.tile([DK, 1024], F32, tag="bcast")
            dn = nrm.tile([1, 1024], F32, tag="denom")
            nc.vector.tensor_copy(out=dn[:], in_=po[DK:DK + 1, :])
            nc.vector.reciprocal_approx_fast(bc[0:1, :], dn[:])
            nc.gpsimd.partition_broadcast(bc[:], bc[0:1, :])
            # write O.T for this (head, q-block) into QT's now-dead region
            nc.vector.tensor_tensor(QTs[jc][pr:pr + DK, qsl], po[0:DK, :], bc[:],
                                    MULT)

    def oproj_qblock(qb):
        # output projection for one q-block (overlaps other work)
        for sc in range(qb * 8, (qb + 1) * 8):
            pso = psB.tile([P, 1024], F32, tag="psB")
            for hd in range(2):
                for ms in range(2):
                    nc.tensor.matmul(
                        pso[:, ms * 512:(ms + 1) * 512],
                        QTs[hd][:, sc * P:(sc + 1) * P],
                        wo_sb[:, hd, ms * 512:(ms + 1) * 512],
                        start=(hd == 0), stop=(hd == 1),
                    )
            ot = outp.tile([P, 1024], F32, tag="osb")
            if qb == 1 and sc % 2 == 1:
                nc.scalar.copy(out=ot[:], in_=pso[:])
            else:
                nc.vector.tensor_copy(out=ot[:], in_=pso[:])
            nc.sync.dma_start(out[sc * P:(sc + 1) * P, :], ot[:])

    attn_qblock(0)
    # Q projection for the second s-half, hidden under qb0 attention; O-proj
    # for qb0 comes after it so qb1's first scores aren't serialized behind it
    proj_block("q", xqT_r, wq_sb, bq_sb, QTs, 1)
    oproj_qblock(0)
    attn_qblock(1)
    oproj_qblock(1)



_prog_cache = {}


def _build_program():
    if "nc" not in _prog_cache:
        from contextlib import ExitStack
        nc = bacc.Bacc("TRN2", target_bir_lowering=False)
        with tile.TileContext(nc) as tc:
            with ExitStack() as ctx:
                _emit(nc, tc, ctx)
        nc.compile()
        _prog_cache["nc"] = nc
    return _prog_cache["nc"]


def make_in_maps(query, key, value, Wq, bq, Wk, bk, Wv, bv, Wo, bo):
    query, key, value = (np.asarray(t, np.float32) for t in (query, key, value))
    Wq, Wk, Wv, Wo = (np.asarray(t, np.float32) for t in (Wq, Wk, Wv, Wo))
    bq, bk, bv = (np.asarray(t, np.float32) for t in (bq, bk, bv))
    xT = {b: {} for b in range(B)}
    for b in range(B):
        xT[b]["q"] = np.ascontiguousarray(query[b].T).astype(np.float16)
        xT[b]["k"] = np.ascontiguousarray(key[b].T).astype(np.float16)
        xT[b]["v"] = np.ascontiguousarray(value[b].T).astype(np.float16)
    in_maps = []
    for c in range(N_CORES):
        b, g = divmod(c, GROUPS)
        gs = slice(g * GW, (g + 1) * GW)
        in_maps.append({
            "xqT": xT[b]["q"], "xkT": xT[b]["k"], "xvT": xT[b]["v"],
            "wqT": np.ascontiguousarray(Wq[gs, :].T).astype(np.float16),
            "wkT": np.ascontiguousarray(Wk[gs, :].T).astype(np.float16),
            "wvT": np.ascontiguousarray(Wv[gs, :].T).astype(np.float16),
            "woT": np.ascontiguousarray(Wo[:, gs].T).astype(np.float16),
            "bq2": np.ascontiguousarray(bq[gs].reshape(2, 128).T),
            "bk2": np.ascontiguousarray(bk[gs].reshape(2, 128).T),
            "bvr": np.ascontiguousarray(bv[gs].reshape(1, GW)),
        })
    return in_maps


def run_on_hw(in_maps, trace=False, **kw):
    nc = _build_program()
    return run_bass_kernel_spmd(nc, in_maps, core_ids=list(range(N_CORES)),
                                trace=trace, **kw)


def kernel(query, key, value, Wq, bq, Wk, bk, Wv, bv, Wo, bo):
    in_maps = make_in_maps(query, key, value, Wq, bq, Wk, bk, Wv, bv, Wo, bo)
    res = run_on_hw(in_maps)
    out = np.zeros((B, S, D_MODEL), np.float32)
    for c in range(N_CORES):
        out[c // GROUPS] += res.results[c]["out"]
    out += np.asarray(bo, np.float32)
    return out


if __name__ == "__main__":
    # self-check against a pure-numpy reference
    rng = np.random.default_rng(0)
    sc = 1.0 / np.sqrt(D_MODEL)
    inp = dict(
        query=rng.standard_normal((B, S, D_MODEL), np.float32),
        key=rng.standard_normal((B, S, D_MODEL), np.float32),
        value=rng.standard_normal((B, S, D_MODEL), np.float32),
        Wq=(rng.standard_normal((D_MODEL, D_MODEL)) * sc).astype(np.float32),
        bq=rng.standard_normal(D_MODEL).astype(np.float32) * 0.1,
        Wk=(rng.standard_normal((D_MODEL, D_MODEL)) * sc).astype(np.float32),
        bk=rng.standard_normal(D_MODEL).astype(np.float32) * 0.1,
        Wv=(rng.standard_normal((D_MODEL, D_MODEL)) * sc).astype(np.float32),
        bv=rng.standard_normal(D_MODEL).astype(np.float32) * 0.1,
        Wo=(rng.standard_normal((D_MODEL, D_MODEL)) * sc).astype(np.float32),
        bo=rng.standard_normal(D_MODEL).astype(np.float32) * 0.1,
    )

    def np_ref(query, key, value, Wq, bq, Wk, bk, Wv, bv, Wo, bo):
        q = query.astype(np.float64) @ Wq.T.astype(np.float64) + bq
        k = key.astype(np.float64) @ Wk.T.astype(np.float64) + bk
        v = value.astype(np.float64) @ Wv.T.astype(np.float64) + bv
        q = q.reshape(B, S, NUM_HEADS, DK).transpose(0, 2, 1, 3)
        k = k.reshape(B, S, NUM_HEADS, DK).transpose(0, 2, 1, 3)
        v = v.reshape(B, S, NUM_HEADS, DK).transpose(0, 2, 1, 3)
        sc_ = np.einsum("bhqd,bhkd->bhqk", q, k) / np.sqrt(DK)
        sc_ -= sc_.max(-1, keepdims=True)
        a = np.exp(sc_)
        a /= a.sum(-1, keepdims=True)
        o = np.einsum("bhqk,bhkd->bhqd", a, v)
        o = o.transpose(0, 2, 1, 3).reshape(B, S, D_MODEL)
        return o @ Wo.T.astype(np.float64) + bo

    exp = np_ref(**inp)
    got = kernel(**inp)
    scale = np.abs(exp).max()
    err = np.abs(got - exp)
    print(f"max abs err {err.max():.4e}  rel {err.max() / scale:.4e}  "
          f"mean rel {err.mean() / scale:.4e}")


# revision 26
# speedup vs baseline: 1.2035x; 1.0455x over previous
"""Multi-head attention (B=2, S=2048, D=1024, H=16) on 8 Trainium2 NeuronCores.

Sharding: tensor-parallel on heads (4 groups of 4 heads) x data-parallel on
batch (2) -> 8 cores. Each core computes QKV projections for its head slice,
attention for its 4 heads, and a partial output projection; the host sums the
4 partials per batch element (the tensor-parallel allreduce) and adds bo.

All matmul operands are fp16 (fp32 PSUM accumulation). Scores are computed
transposed (ST[k,q] = KT_h.T @ QT_h) so softmax exp feeds attn@V directly as
the stationary operand with no transposes; a ones-column appended to V makes
the same matmul accumulate the softmax denominators.
"""

import numpy as np

import concourse.bass as bass  # noqa: F401
import concourse.tile as tile
from concourse import bacc, mybir
from concourse.bass_utils import run_bass_kernel_spmd
D_MODEL = 1024
NUM_HEADS = 16
DK = 64
B, S = 2, 2048
N_CORES = 8
GROUPS = 4                 # head groups (tensor parallel)
GW = D_MODEL // GROUPS     # 256 features per group = 4 heads
HPG = GROUPS               # heads per group = 4

F32 = mybir.dt.float32
BF16 = mybir.dt.float16  # 16-bit matmul operand dtype
EXPF = mybir.ActivationFunctionType.Exp
MULT = mybir.AluOpType.mult
ADD = mybir.AluOpType.add


def _emit(nc, tc, ctx):
    P = 128
    xqT = nc.dram_tensor("xqT", [D_MODEL, S], BF16, kind="ExternalInput")
    xkT = nc.dram_tensor("xkT", [D_MODEL, S], BF16, kind="ExternalInput")
    xvT = nc.dram_tensor("xvT", [D_MODEL, S], BF16, kind="ExternalInput")
    wqT = nc.dram_tensor("wqT", [D_MODEL, GW], BF16, kind="ExternalInput")
    wkT = nc.dram_tensor("wkT", [D_MODEL, GW], BF16, kind="ExternalInput")
    wvT = nc.dram_tensor("wvT", [D_MODEL, GW], BF16, kind="ExternalInput")
    woT = nc.dram_tensor("woT", [GW, D_MODEL], BF16, kind="ExternalInput")
    bq2 = nc.dram_tensor("bq2", [P, 2], F32, kind="ExternalInput")
    bk2 = nc.dram_tensor("bk2", [P, 2], F32, kind="ExternalInput")
    bvr = nc.dram_tensor("bvr", [1, GW], F32, kind="ExternalInput")
    out = nc.dram_tensor("out", [S, D_MODEL], F32, kind="ExternalOutput")

    consts = ctx.enter_context(tc.tile_pool(name="consts", bufs=1))
    persist = ctx.enter_context(tc.tile_pool(name="persist", bufs=1))
    xs = ctx.enter_context(tc.tile_pool(name="xs", bufs=5))
    sx = ctx.enter_context(tc.tile_pool(name="stexp", bufs=3))
    nrm = ctx.enter_context(tc.tile_pool(name="nrm", bufs=2))
    outp = ctx.enter_context(tc.tile_pool(name="outp", bufs=3))
    psA = ctx.enter_context(tc.tile_pool(name="psA", bufs=2, space="PSUM"))
    psB = ctx.enter_context(tc.tile_pool(name="psB", bufs=2, space="PSUM"))

    # ---- constants / weights -------------------------------------------
    wq_sb = consts.tile([P, 8, GW], BF16)
    wk_sb = consts.tile([P, 8, GW], BF16)
    wv_sb = consts.tile([P, 8, GW], BF16)
    wo_sb = consts.tile([P, 2, D_MODEL], BF16)
    nc.sync.dma_start(wq_sb[:], wqT[:].rearrange("(c p) j -> p c j", p=P))
    bq_sb = consts.tile([P, 2], F32)
    bk_sb = consts.tile([P, 2], F32)
    nc.sync.dma_start(bq_sb[:], bq2[:])
    nc.sync.dma_start(bk_sb[:], bk2[:])
    bv_row = consts.tile([1, GW], F32)
    nc.sync.dma_start(bv_row[:], bvr[:])
    bvb = consts.tile([P, GW], F32)
    nc.gpsimd.partition_broadcast(bvb[:], bv_row[:])

    # persistent activations (QT doubles as O.T after attention), split by
    # feature chunk / s-half so attention can start before phase 1 finishes
    QTs = [persist.tile([P, S], BF16, name=f"QT{j}") for j in range(2)]
    # KT split by (feature chunk, s-half): scores over k<1024 need only K(sb0)
    KT4 = [[persist.tile([P, 1024], BF16, name=f"KT{j}_{hh}") for hh in range(2)]
           for j in range(2)]
    Vaugs = [persist.tile([P, 8, HPG, DK + 1], BF16, name=f"Vaug{v}")
             for v in range(2)]
    ones_f32 = consts.tile([P, 8, HPG], F32)
    nc.vector.memset(ones_f32[:], 1.0)
    for v in range(2):
        nc.vector.tensor_scalar_add(Vaugs[v][:, :, :, DK], ones_f32[:], 0.0)

    xqT_r = xqT[:].rearrange("(c p) s -> p c s", p=P)
    xkT_r = xkT[:].rearrange("(c p) s -> p c s", p=P)
    xvT_r = xvT[:].rearrange("(c p) s -> p c s", p=P)

    # ---- phase 1: QKV projections (Q fully, then K, then V) -------------
    def proj_block(name, x_r, w_sb, b_sb, dstTs, sb, pre=None):
        ssl = slice(sb * 1024, (sb + 1) * 1024)
        ps = [psA.tile([P, 1024], F32, tag="psA", name=f"ps_{name}{sb}{j}")
              for j in range(2)]
        for i in range(8):
            if pre is not None:
                xt = pre[:, i, :]
            else:
                xt = xs.tile([P, 1024], BF16, tag="xs")
                nc.sync.dma_start(xt[:], x_r[:, i, ssl])
            for j in range(2):
                for ns in range(2):
                    nc.tensor.matmul(
                        ps[j][:, ns * 512:(ns + 1) * 512],
                        w_sb[:, i, j * P:(j + 1) * P],
                        xt[:, ns * 512:(ns + 1) * 512],
                        start=(i == 0), stop=(i == 7),
                    )
        for j in range(2):
            if name == "k":
                nc.vector.tensor_scalar_add(dstTs[j][sb][:, :], ps[j][:],
                                            b_sb[:, j:j + 1])
            else:
                nc.vector.tensor_scalar_add(dstTs[j][:, ssl], ps[j][:],
                                            b_sb[:, j:j + 1])

    nc.sync.dma_start(wk_sb[:], wkT[:].rearrange("(c p) j -> p c j", p=P))
    proj_block("q", xqT_r, wq_sb, bq_sb, QTs, 0)
    nc.sync.dma_start(wv_sb[:], wvT[:].rearrange("(c p) j -> p c j", p=P))
    proj_block("k", xkT_r, wk_sb, bk_sb, KT4, 0)
    proj_block("k", xkT_r, wk_sb, bk_sb, KT4, 1)
    xq1 = persist.tile([P, 8, 1024], BF16, name="xq1")
    nc.sync.dma_start(xq1[:], xqT_r[:, :, 1024:2048])
    nc.sync.dma_start(wo_sb[:], woT[:].rearrange("(c p) m -> p c m", p=P))
    # V: natural layout, s on partitions
    for g_ss in range(16):
        xvt = xs.tile([P, 8, P], BF16, tag="xs", name=f"xv{g_ss}")
        nc.sync.dma_start(xvt[:], xvT_r[:, :, g_ss * P:(g_ss + 1) * P])
        pv = psB.tile([P, 1024], F32, tag="psB")
        for i in range(8):
            nc.tensor.matmul(
                pv[:, 0:GW], xvt[:, i, :], wv_sb[:, i, :],
                start=(i == 0), stop=(i == 7),
            )
        nc.vector.tensor_tensor(
            Vaugs[g_ss // 8][:, g_ss % 8, :, 0:DK],
            pv[:, 0:GW].rearrange("p (h d) -> p h d", h=HPG),
            bvb[:].rearrange("p (h d) -> p h d", h=HPG),
            ADD,
        )

    # ---- phase 2: attention per (head, q-block) ------------------------
    def attn_qblock(qb, defer=False):
        for h in range(HPG):
            pr = 64 * (h % 2)   # partition offset of this head's features
            jc = h // 2         # feature chunk
            qsl = slice(qb * 1024, (qb + 1) * 1024)
            st = sx.tile([P, 16, 1024], BF16, tag="stexp")
            po = psB.tile([P, 1024], F32, tag="psB")
            for k in range(16):
                pst = psA.tile([P, 1024], F32, tag="psA")
                for ns in range(2):
                    nc.tensor.matmul(
                        pst[:, ns * 512:(ns + 1) * 512],
                        KT4[jc][k // 8][pr:pr + DK, (k % 8) * P:(k % 8 + 1) * P],
                        QTs[jc][pr:pr + DK, qb * 1024 + ns * 512:
                                qb * 1024 + (ns + 1) * 512],
                        start=True, stop=True,
                    )
                nc.scalar.activation(out=st[:, k, :], in_=pst[:], func=EXPF,
                                     scale=0.125)
                for ns in range(2):
                    nc.tensor.matmul(
                        po[0:DK + 1, ns * 512:(ns + 1) * 512],
                        Vaugs[k // 8][:, k % 8, h, :],
                        st[:, k, ns * 512:(ns + 1) * 512],
                        start=(k == 0), stop=(k == 15),
                    )
            # normalize: row DK of po holds softmax denominators
            bc = nrm.tile([DK, 1024], F32, tag="bcast")
            dn = nrm.tile([1, 1024], F32, tag="denom")
            nc.vector.tensor_copy(out=dn[:], in_=po[DK:DK + 1, :])
            nc.vector.reciprocal_approx_fast(bc[0:1, :], dn[:])
            nc.gpsimd.partition_broadcast(bc[:], bc[0:1, :])
            # write O.T for this (head, q-block) into QT's now-dead region
            nc.vector.tensor_tensor(QTs[jc][pr:pr + DK, qsl], po[0:DK, :], bc[:],
                                    MULT)

    def oproj_qblock(qb):
        # output projection for one q-block (overlaps other work)
        for sc in range(qb * 8, (qb + 1) * 8):
            pso = psB.tile([P, 1024], F32, tag="psB")
            for hd in range(2):
                for ms in range(2):
                    nc.tensor.matmul(
                        pso[:, ms * 512:(ms + 1) * 512],
                        QTs[hd][:, sc * P:(sc + 1) * P],
                        wo_sb[:, hd, ms * 512:(ms + 1) * 512],
                        start=(hd == 0), stop=(hd == 1),
                    )
            ot = outp.tile([P, 1024], F32, tag="osb")
            if qb == 1 and sc % 2 == 1:
                nc.scalar.copy(out=ot[:], in_=pso[:])
            else:
                nc.vector.tensor_copy(out=ot[:], in_=pso[:])
            nc.sync.dma_start(out[sc * P:(sc + 1) * P, :], ot[:])

    attn_qblock(0)
    # Q projection for the second s-half (input prefetched into xq1); O-proj
    # for qb0 comes after it so qb1's first scores aren't serialized behind it
    proj_block("q", xqT_r, wq_sb, bq_sb, QTs, 1, pre=xq1)
    oproj_qblock(0)
    attn_qblock(1)
    oproj_qblock(1)



_prog_cache = {}


def _build_program():
    if "nc" not in _prog_cache:
        from contextlib import ExitStack
        nc = bacc.Bacc("TRN2", target_bir_lowering=False)
        with tile.TileContext(nc) as tc:
            with ExitStack() as ctx:
                _emit(nc, tc, ctx)
        nc.compile()
        _prog_cache["nc"] = nc
    return _prog_cache["nc"]


def make_in_maps(query, key, value, Wq, bq, Wk, bk, Wv, bv, Wo, bo):
    query, key, value = (np.asarray(t, np.float32) for t in (query, key, value))
    Wq, Wk, Wv, Wo = (np.asarray(t, np.float32) for t in (Wq, Wk, Wv, Wo))
    bq, bk, bv = (np.asarray(t, np.float32) for t in (bq, bk, bv))
    xT = {b: {} for b in range(B)}
    for b in range(B):
        xT[b]["q"] = np.ascontiguousarray(query[b].T).astype(np.float16)
        xT[b]["k"] = np.ascontiguousarray(key[b].T).astype(np.float16)
        xT[b]["v"] = np.ascontiguousarray(value[b].T).astype(np.float16)
    in_maps = []
    for c in range(N_CORES):
        b, g = divmod(c, GROUPS)
        gs = slice(g * GW, (g + 1) * GW)
        in_maps.append({
            "xqT": xT[b]["q"], "xkT": xT[b]["k"], "xvT": xT[b]["v"],
            "wqT": np.ascontiguousarray(Wq[gs, :].T).astype(np.float16),
            "wkT": np.ascontiguousarray(Wk[gs, :].T).astype(np.float16),
            "wvT": np.ascontiguousarray(Wv[gs, :].T).astype(np.float16),
            "woT": np.ascontiguousarray(Wo[:, gs].T).astype(np.float16),
            "bq2": np.ascontiguousarray(bq[gs].reshape(2, 128).T),
            "bk2": np.ascontiguousarray(bk[gs].reshape(2, 128).T),
            "bvr": np.ascontiguousarray(bv[gs].reshape(1, GW)),
        })
    return in_maps


def run_on_hw(in_maps, trace=False, **kw):
    nc = _build_program()
    return run_bass_kernel_spmd(nc, in_maps, core_ids=list(range(N_CORES)),
                                trace=trace, **kw)


def kernel(query, key, value, Wq, bq, Wk, bk, Wv, bv, Wo, bo):
    in_maps = make_in_maps(query, key, value, Wq, bq, Wk, bk, Wv, bv, Wo, bo)
    res = run_on_hw(in_maps)
    out = np.zeros((B, S, D_MODEL), np.float32)
    for c in range(N_CORES):
        out[c // GROUPS] += res.results[c]["out"]
    out += np.asarray(bo, np.float32)
    return out


if __name__ == "__main__":
    # self-check against a pure-numpy reference
    rng = np.random.default_rng(0)
    sc = 1.0 / np.sqrt(D_MODEL)
    inp = dict(
        query=rng.standard_normal((B, S, D_MODEL), np.float32),
        key=rng.standard_normal((B, S, D_MODEL), np.float32),
        value=rng.standard_normal((B, S, D_MODEL), np.float32),
        Wq=(rng.standard_normal((D_MODEL, D_MODEL)) * sc).astype(np.float32),
        bq=rng.standard_normal(D_MODEL).astype(np.float32) * 0.1,
        Wk=(rng.standard_normal((D_MODEL, D_MODEL)) * sc).astype(np.float32),
        bk=rng.standard_normal(D_MODEL).astype(np.float32) * 0.1,
        Wv=(rng.standard_normal((D_MODEL, D_MODEL)) * sc).astype(np.float32),
        bv=rng.standard_normal(D_MODEL).astype(np.float32) * 0.1,
        Wo=(rng.standard_normal((D_MODEL, D_MODEL)) * sc).astype(np.float32),
        bo=rng.standard_normal(D_MODEL).astype(np.float32) * 0.1,
    )

    def np_ref(query, key, value, Wq, bq, Wk, bk, Wv, bv, Wo, bo):
        q = query.astype(np.float64) @ Wq.T.astype(np.float64) + bq
        k = key.astype(np.float64) @ Wk.T.astype(np.float64) + bk
        v = value.astype(np.float64) @ Wv.T.astype(np.float64) + bv
        q = q.reshape(B, S, NUM_HEADS, DK).transpose(0, 2, 1, 3)
        k = k.reshape(B, S, NUM_HEADS, DK).transpose(0, 2, 1, 3)
        v = v.reshape(B, S, NUM_HEADS, DK).transpose(0, 2, 1, 3)
        sc_ = np.einsum("bhqd,bhkd->bhqk", q, k) / np.sqrt(DK)
        sc_ -= sc_.max(-1, keepdims=True)
        a = np.exp(sc_)
        a /= a.sum(-1, keepdims=True)
        o = np.einsum("bhqk,bhkd->bhqd", a, v)
        o = o.transpose(0, 2, 1, 3).reshape(B, S, D_MODEL)
        return o @ Wo.T.astype(np.float64) + bo

    exp = np_ref(**inp)
    got = kernel(**inp)
    scale = np.abs(exp).max()
    err = np.abs(got - exp)
    print(f"max abs err {err.max():.4e}  rel {err.max() / scale:.4e}  "
          f"mean rel {err.mean() / scale:.4e}")


# revision 27
# speedup vs baseline: 1.2064x; 1.0024x over previous
"""Multi-head attention (B=2, S=2048, D=1024, H=16) on 8 Trainium2 NeuronCores.

Sharding: tensor-parallel on heads (4 groups of 4 heads) x data-parallel on
batch (2) -> 8 cores. Each core computes QKV projections for its head slice,
attention for its 4 heads, and a partial output projection; the host sums the
4 partials per batch element (the tensor-parallel allreduce) and adds bo.

All matmul operands are fp16 (fp32 PSUM accumulation). Scores are computed
transposed (ST[k,q] = KT_h.T @ QT_h) so softmax exp feeds attn@V directly as
the stationary operand with no transposes; a ones-column appended to V makes
the same matmul accumulate the softmax denominators.
"""

import numpy as np

import concourse.bass as bass  # noqa: F401
import concourse.tile as tile
from concourse import bacc, mybir
from concourse.bass_utils import run_bass_kernel_spmd
D_MODEL = 1024
NUM_HEADS = 16
DK = 64
B, S = 2, 2048
N_CORES = 8
GROUPS = 4                 # head groups (tensor parallel)
GW = D_MODEL // GROUPS     # 256 features per group = 4 heads
HPG = GROUPS               # heads per group = 4

F32 = mybir.dt.float32
BF16 = mybir.dt.float16  # 16-bit matmul operand dtype
EXPF = mybir.ActivationFunctionType.Exp
MULT = mybir.AluOpType.mult
ADD = mybir.AluOpType.add


def _emit(nc, tc, ctx):
    P = 128
    xqT = nc.dram_tensor("xqT", [D_MODEL, S], BF16, kind="ExternalInput")
    xkT = nc.dram_tensor("xkT", [D_MODEL, S], BF16, kind="ExternalInput")
    xvT = nc.dram_tensor("xvT", [D_MODEL, S], BF16, kind="ExternalInput")
    wqT = nc.dram_tensor("wqT", [D_MODEL, GW], BF16, kind="ExternalInput")
    wkT = nc.dram_tensor("wkT", [D_MODEL, GW], BF16, kind="ExternalInput")
    wvT = nc.dram_tensor("wvT", [D_MODEL, GW], BF16, kind="ExternalInput")
    woT = nc.dram_tensor("woT", [GW, D_MODEL], BF16, kind="ExternalInput")
    bq2 = nc.dram_tensor("bq2", [P, 2], F32, kind="ExternalInput")
    bk2 = nc.dram_tensor("bk2", [P, 2], F32, kind="ExternalInput")
    bvr = nc.dram_tensor("bvr", [1, GW], F32, kind="ExternalInput")
    out = nc.dram_tensor("out", [S, D_MODEL], F32, kind="ExternalOutput")

    consts = ctx.enter_context(tc.tile_pool(name="consts", bufs=1))
    persist = ctx.enter_context(tc.tile_pool(name="persist", bufs=1))
    xs = ctx.enter_context(tc.tile_pool(name="xs", bufs=5))
    sx = ctx.enter_context(tc.tile_pool(name="stexp", bufs=3))
    nrm = ctx.enter_context(tc.tile_pool(name="nrm", bufs=2))
    outp = ctx.enter_context(tc.tile_pool(name="outp", bufs=3))
    psA = ctx.enter_context(tc.tile_pool(name="psA", bufs=2, space="PSUM"))
    psB = ctx.enter_context(tc.tile_pool(name="psB", bufs=2, space="PSUM"))

    # ---- constants / weights -------------------------------------------
    wq_sb = consts.tile([P, 8, GW], BF16)
    wk_sb = consts.tile([P, 8, GW], BF16)
    wv_sb = consts.tile([P, 8, GW], BF16)
    wo_sb = consts.tile([P, 2, D_MODEL], BF16)
    nc.sync.dma_start(wq_sb[:], wqT[:].rearrange("(c p) j -> p c j", p=P))
    bq_sb = consts.tile([P, 2], F32)
    bk_sb = consts.tile([P, 2], F32)
    nc.sync.dma_start(bq_sb[:], bq2[:])
    nc.sync.dma_start(bk_sb[:], bk2[:])
    bv_row = consts.tile([1, GW], F32)
    nc.sync.dma_start(bv_row[:], bvr[:])
    bvb = consts.tile([P, GW], F32)
    nc.gpsimd.partition_broadcast(bvb[:], bv_row[:])

    # persistent activations (QT doubles as O.T after attention), split by
    # feature chunk / s-half so attention can start before phase 1 finishes
    QTs = [persist.tile([P, S], BF16, name=f"QT{j}") for j in range(2)]
    # KT split by (feature chunk, s-half): scores over k<1024 need only K(sb0)
    KT4 = [[persist.tile([P, 1024], BF16, name=f"KT{j}_{hh}") for hh in range(2)]
           for j in range(2)]
    Vaugs = [persist.tile([P, 8, HPG, DK + 1], BF16, name=f"Vaug{v}")
             for v in range(2)]
    ones_f32 = consts.tile([P, 8, HPG], F32)
    nc.vector.memset(ones_f32[:], 1.0)
    for v in range(2):
        nc.vector.tensor_scalar_add(Vaugs[v][:, :, :, DK], ones_f32[:], 0.0)

    xqT_r = xqT[:].rearrange("(c p) s -> p c s", p=P)
    xkT_r = xkT[:].rearrange("(c p) s -> p c s", p=P)
    xvT_r = xvT[:].rearrange("(c p) s -> p c s", p=P)

    # ---- phase 1: QKV projections (Q fully, then K, then V) -------------
    def proj_block(name, x_r, w_sb, b_sb, dstTs, sb, pre=None):
        ssl = slice(sb * 1024, (sb + 1) * 1024)
        ps = [psA.tile([P, 1024], F32, tag="psA", name=f"ps_{name}{sb}{j}")
              for j in range(2)]
        for i in range(8):
            if pre is not None:
                xt = pre[:, i, :]
            else:
                xt = xs.tile([P, 1024], BF16, tag="xs")
                nc.sync.dma_start(xt[:], x_r[:, i, ssl])
            for j in range(2):
                for ns in range(2):
                    nc.tensor.matmul(
                        ps[j][:, ns * 512:(ns + 1) * 512],
                        w_sb[:, i, j * P:(j + 1) * P],
                        xt[:, ns * 512:(ns + 1) * 512],
                        start=(i == 0), stop=(i == 7),
                    )
        for j in range(2):
            if name == "k":
                nc.vector.tensor_scalar_add(dstTs[j][sb][:, :], ps[j][:],
                                            b_sb[:, j:j + 1])
            else:
                nc.vector.tensor_scalar_add(dstTs[j][:, ssl], ps[j][:],
                                            b_sb[:, j:j + 1])

    nc.sync.dma_start(wk_sb[:], wkT[:].rearrange("(c p) j -> p c j", p=P))
    proj_block("q", xqT_r, wq_sb, bq_sb, QTs, 0)
    nc.sync.dma_start(wv_sb[:], wvT[:].rearrange("(c p) j -> p c j", p=P))
    proj_block("k", xkT_r, wk_sb, bk_sb, KT4, 0)
    proj_block("k", xkT_r, wk_sb, bk_sb, KT4, 1)
    nc.sync.dma_start(wo_sb[:], woT[:].rearrange("(c p) m -> p c m", p=P))
    # V: natural layout, s on partitions
    for g_ss in range(16):
        xvt = xs.tile([P, 8, P], BF16, tag="xs", name=f"xv{g_ss}")
        nc.sync.dma_start(xvt[:], xvT_r[:, :, g_ss * P:(g_ss + 1) * P])
        pv = psB.tile([P, 1024], F32, tag="psB")
        for i in range(8):
            nc.tensor.matmul(
                pv[:, 0:GW], xvt[:, i, :], wv_sb[:, i, :],
                start=(i == 0), stop=(i == 7),
            )
        nc.vector.tensor_tensor(
            Vaugs[g_ss // 8][:, g_ss % 8, :, 0:DK],
            pv[:, 0:GW].rearrange("p (h d) -> p h d", h=HPG),
            bvb[:].rearrange("p (h d) -> p h d", h=HPG),
            ADD,
        )

    # ---- phase 2: attention per (head, q-block) ------------------------
    def attn_qblock(qb, defer=False):
        for h in range(HPG):
            pr = 64 * (h % 2)   # partition offset of this head's features
            jc = h // 2         # feature chunk
            qsl = slice(qb * 1024, (qb + 1) * 1024)
            st = sx.tile([P, 16, 1024], BF16, tag="stexp")
            po = psB.tile([P, 1024], F32, tag="psB")
            for k in range(16):
                pst = psA.tile([P, 1024], F32, tag="psA")
                for ns in range(2):
                    nc.tensor.matmul(
                        pst[:, ns * 512:(ns + 1) * 512],
                        KT4[jc][k // 8][pr:pr + DK, (k % 8) * P:(k % 8 + 1) * P],
                        QTs[jc][pr:pr + DK, qb * 1024 + ns * 512:
                                qb * 1024 + (ns + 1) * 512],
                        start=True, stop=True,
                    )
                nc.scalar.activation(out=st[:, k, :], in_=pst[:], func=EXPF,
                                     scale=0.125)
                for ns in range(2):
                    nc.tensor.matmul(
                        po[0:DK + 1, ns * 512:(ns + 1) * 512],
                        Vaugs[k // 8][:, k % 8, h, :],
                        st[:, k, ns * 512:(ns + 1) * 512],
                        start=(k == 0), stop=(k == 15),
                    )
            # normalize: row DK of po holds softmax denominators
            bc = nrm.tile([DK, 1024], F32, tag="bcast")
            dn = nrm.tile([1, 1024], F32, tag="denom")
            nc.vector.tensor_copy(out=dn[:], in_=po[DK:DK + 1, :])
            nc.vector.reciprocal_approx_fast(bc[0:1, :], dn[:])
            nc.gpsimd.partition_broadcast(bc[:], bc[0:1, :])
            # write O.T for this (head, q-block) into QT's now-dead region
            nc.vector.tensor_tensor(QTs[jc][pr:pr + DK, qsl], po[0:DK, :], bc[:],
                                    MULT)

    def oproj_qblock(qb):
        # output projection for one q-block (overlaps other work)
        for sc in range(qb * 8, (qb + 1) * 8):
            pso = psB.tile([P, 1024], F32, tag="psB")
            for hd in range(2):
                for ms in range(2):
                    nc.tensor.matmul(
                        pso[:, ms * 512:(ms + 1) * 512],
                        QTs[hd][:, sc * P:(sc + 1) * P],
                        wo_sb[:, hd, ms * 512:(ms + 1) * 512],
                        start=(hd == 0), stop=(hd == 1),
                    )
            ot = outp.tile([P, 1024], F32, tag="osb")
            if qb == 1 and sc % 2 == 1:
                nc.scalar.copy(out=ot[:], in_=pso[:])
            else:
                nc.vector.tensor_copy(out=ot[:], in_=pso[:])
            nc.sync.dma_start(out[sc * P:(sc + 1) * P, :], ot[:])

    attn_qblock(0)
    # Q projection for the second s-half, hidden under qb0 attention; O-proj
    # for qb0 comes after it so qb1's first scores aren't serialized behind it
    proj_block("q", xqT_r, wq_sb, bq_sb, QTs, 1)
    oproj_qblock(0)
    attn_qblock(1)
    oproj_qblock(1)



_prog_cache = {}


def _build_program():
    if "nc" not in _prog_cache:
        from contextlib import ExitStack
        nc = bacc.Bacc("TRN2", target_bir_lowering=False)
        with tile.TileContext(nc) as tc:
            with ExitStack() as ctx:
                _emit(nc, tc, ctx)
        nc.compile()
        _prog_cache["nc"] = nc
    return _prog_cache["nc"]


def make_in_maps(query, key, value, Wq, bq, Wk, bk, Wv, bv, Wo, bo):
    query, key, value = (np.asarray(t, np.float32) for t in (query, key, value))
    Wq, Wk, Wv, Wo = (np.asarray(t, np.float32) for t in (Wq, Wk, Wv, Wo))
    bq, bk, bv = (np.asarray(t, np.float32) for t in (bq, bk, bv))
    xT = {b: {} for b in range(B)}
    for b in range(B):
        xT[b]["q"] = np.ascontiguousarray(query[b].T).astype(np.float16)
        xT[b]["k"] = np.ascontiguousarray(key[b].T).astype(np.float16)
        xT[b]["v"] = np.ascontiguousarray(value[b].T).astype(np.float16)
    in_maps = []
    for c in range(N_CORES):
        b, g = divmod(c, GROUPS)
        gs = slice(g * GW, (g + 1) * GW)
        in_maps.append({
            "xqT": xT[b]["q"], "xkT": xT[b]["k"], "xvT": xT[b]["v"],
            "wqT": np.ascontiguousarray(Wq[gs, :].T).astype(np.float16),
            "wkT": np.ascontiguousarray(Wk[gs, :].T).astype(np.float16),
            "wvT": np.ascontiguousarray(Wv[gs, :].T).astype(np.float16),
            "woT": np.ascontiguousarray(Wo[:, gs].T).astype(np.float16),
            "bq2": np.ascontiguousarray(bq[gs].reshape(2, 128).T),
            "bk2": np.ascontiguousarray(bk[gs].reshape(2, 128).T),
            "bvr": np.ascontiguousarray(bv[gs].reshape(1, GW)),
        })
    return in_maps


def run_on_hw(in_maps, trace=False, **kw):
    nc = _build_program()
    return run_bass_kernel_spmd(nc, in_maps, core_ids=list(range(N_CORES)),
                                trace=trace, **kw)


def kernel(query, key, value, Wq, bq, Wk, bk, Wv, bv, Wo, bo):
    in_maps = make_in_maps(query, key, value, Wq, bq, Wk, bk, Wv, bv, Wo, bo)
    res = run_on_hw(in_maps)
    out = np.zeros((B, S, D_MODEL), np.float32)
    for c in range(N_CORES):
        out[c // GROUPS] += res.results[c]["out"]
    out += np.asarray(bo, np.float32)
    return out


if __name__ == "__main__":
    # self-check against a pure-numpy reference
    rng = np.random.default_rng(0)
    sc = 1.0 / np.sqrt(D_MODEL)
    inp = dict(
        query=rng.standard_normal((B, S, D_MODEL), np.float32),
        key=rng.standard_normal((B, S, D_MODEL), np.float32),
        value=rng.standard_normal((B, S, D_MODEL), np.float32),
        Wq=(rng.standard_normal((D_MODEL, D_MODEL)) * sc).astype(np.float32),
        bq=rng.standard_normal(D_MODEL).astype(np.float32) * 0.1,
        Wk=(rng.standard_normal((D_MODEL, D_MODEL)) * sc).astype(np.float32),
        bk=rng.standard_normal(D_MODEL).astype(np.float32) * 0.1,
        Wv=(rng.standard_normal((D_MODEL, D_MODEL)) * sc).astype(np.float32),
        bv=rng.standard_normal(D_MODEL).astype(np.float32) * 0.1,
        Wo=(rng.standard_normal((D_MODEL, D_MODEL)) * sc).astype(np.float32),
        bo=rng.standard_normal(D_MODEL).astype(np.float32) * 0.1,
    )

    def np_ref(query, key, value, Wq, bq, Wk, bk, Wv, bv, Wo, bo):
        q = query.astype(np.float64) @ Wq.T.astype(np.float64) + bq
        k = key.astype(np.float64) @ Wk.T.astype(np.float64) + bk
        v = value.astype(np.float64) @ Wv.T.astype(np.float64) + bv
        q = q.reshape(B, S, NUM_HEADS, DK).transpose(0, 2, 1, 3)
        k = k.reshape(B, S, NUM_HEADS, DK).transpose(0, 2, 1, 3)
        v = v.reshape(B, S, NUM_HEADS, DK).transpose(0, 2, 1, 3)
        sc_ = np.einsum("bhqd,bhkd->bhqk", q, k) / np.sqrt(DK)
        sc_ -= sc_.max(-1, keepdims=True)
        a = np.exp(sc_)
        a /= a.sum(-1, keepdims=True)
        o = np.einsum("bhqk,bhkd->bhqd", a, v)
        o = o.transpose(0, 2, 1, 3).reshape(B, S, D_MODEL)
        return o @ Wo.T.astype(np.float64) + bo

    exp = np_ref(**inp)
    got = kernel(**inp)
    scale = np.abs(exp).max()
    err = np.abs(got - exp)
    print(f"max abs err {err.max():.4e}  rel {err.max() / scale:.4e}  "
          f"mean rel {err.mean() / scale:.4e}")


# revision 28
# speedup vs baseline: 1.2238x; 1.0144x over previous
"""Multi-head attention (B=2, S=2048, D=1024, H=16) on 8 Trainium2 NeuronCores.

Sharding: tensor-parallel on heads (4 groups of 4 heads) x data-parallel on
batch (2) -> 8 cores. Each core computes QKV projections for its head slice,
attention for its 4 heads, and a partial output projection; the host sums the
4 partials per batch element (the tensor-parallel allreduce) and adds bo.

All matmul operands are fp16 (fp32 PSUM accumulation). Scores are computed
transposed (ST[k,q] = KT_h.T @ QT_h) so softmax exp feeds attn@V directly as
the stationary operand with no transposes; a ones-column appended to V makes
the same matmul accumulate the softmax denominators.
"""

import numpy as np

import concourse.bass as bass  # noqa: F401
import concourse.tile as tile
from concourse import bacc, mybir
from concourse.bass_utils import run_bass_kernel_spmd
D_MODEL = 1024
NUM_HEADS = 16
DK = 64
B, S = 2, 2048
N_CORES = 8
GROUPS = 4                 # head groups (tensor parallel)
GW = D_MODEL // GROUPS     # 256 features per group = 4 heads
HPG = GROUPS               # heads per group = 4

F32 = mybir.dt.float32
BF16 = mybir.dt.float16  # 16-bit matmul operand dtype
EXPF = mybir.ActivationFunctionType.Exp
MULT = mybir.AluOpType.mult
ADD = mybir.AluOpType.add


def _emit(nc, tc, ctx):
    P = 128
    xqT = nc.dram_tensor("xqT", [D_MODEL, S], BF16, kind="ExternalInput")
    xkT = nc.dram_tensor("xkT", [D_MODEL, S], BF16, kind="ExternalInput")
    xvT = nc.dram_tensor("xvT", [D_MODEL, S], BF16, kind="ExternalInput")
    wqT = nc.dram_tensor("wqT", [D_MODEL, GW], BF16, kind="ExternalInput")
    wkT = nc.dram_tensor("wkT", [D_MODEL, GW], BF16, kind="ExternalInput")
    wvT = nc.dram_tensor("wvT", [D_MODEL, GW], BF16, kind="ExternalInput")
    woT = nc.dram_tensor("woT", [GW, D_MODEL], BF16, kind="ExternalInput")
    bq2 = nc.dram_tensor("bq2", [P, 2], F32, kind="ExternalInput")
    bk2 = nc.dram_tensor("bk2", [P, 2], F32, kind="ExternalInput")
    bvr = nc.dram_tensor("bvr", [1, GW], F32, kind="ExternalInput")
    out = nc.dram_tensor("out", [S, D_MODEL], F32, kind="ExternalOutput")

    consts = ctx.enter_context(tc.tile_pool(name="consts", bufs=1))
    persist = ctx.enter_context(tc.tile_pool(name="persist", bufs=1))
    xs = ctx.enter_context(tc.tile_pool(name="xs", bufs=5))
    sx = ctx.enter_context(tc.tile_pool(name="stexp", bufs=3))
    nrm = ctx.enter_context(tc.tile_pool(name="nrm", bufs=3))
    outp = ctx.enter_context(tc.tile_pool(name="outp", bufs=4))
    psA = ctx.enter_context(tc.tile_pool(name="psA", bufs=2, space="PSUM"))
    psB = ctx.enter_context(tc.tile_pool(name="psB", bufs=2, space="PSUM"))

    # ---- constants / weights -------------------------------------------
    wq_sb = consts.tile([P, 8, GW], BF16)
    wk_sb = consts.tile([P, 8, GW], BF16)
    wv_sb = consts.tile([P, 8, GW], BF16)
    wo_sb = consts.tile([P, 2, D_MODEL], BF16)
    nc.sync.dma_start(wq_sb[:], wqT[:].rearrange("(c p) j -> p c j", p=P))
    bq_sb = consts.tile([P, 2], F32)
    bk_sb = consts.tile([P, 2], F32)
    nc.sync.dma_start(bq_sb[:], bq2[:])
    nc.sync.dma_start(bk_sb[:], bk2[:])
    bv_row = consts.tile([1, GW], F32)
    nc.sync.dma_start(bv_row[:], bvr[:])
    bvb = consts.tile([P, GW], F32)
    nc.gpsimd.partition_broadcast(bvb[:], bv_row[:])

    # persistent activations (QT doubles as O.T after attention), split by
    # feature chunk / s-half so attention can start before phase 1 finishes
    QTs = [persist.tile([P, S], BF16, name=f"QT{j}") for j in range(2)]
    # KT split by (feature chunk, s-half): scores over k<1024 need only K(sb0)
    KT4 = [[persist.tile([P, 1024], BF16, name=f"KT{j}_{hh}") for hh in range(2)]
           for j in range(2)]
    Vaugs = [persist.tile([P, 8, HPG, DK + 1], BF16, name=f"Vaug{v}")
             for v in range(2)]
    ones_f32 = consts.tile([P, 8, HPG], F32)
    nc.vector.memset(ones_f32[:], 1.0)
    for v in range(2):
        nc.vector.tensor_scalar_add(Vaugs[v][:, :, :, DK], ones_f32[:], 0.0)

    xqT_r = xqT[:].rearrange("(c p) s -> p c s", p=P)
    xkT_r = xkT[:].rearrange("(c p) s -> p c s", p=P)
    xvT_r = xvT[:].rearrange("(c p) s -> p c s", p=P)

    # ---- phase 1: QKV projections (Q fully, then K, then V) -------------
    def proj_block(name, x_r, w_sb, b_sb, dstTs, sb, pre=None):
        ssl = slice(sb * 1024, (sb + 1) * 1024)
        ps = [psA.tile([P, 1024], F32, tag="psA", name=f"ps_{name}{sb}{j}")
              for j in range(2)]
        for i in range(8):
            if pre is not None:
                xt = pre[:, i, :]
            else:
                xt = xs.tile([P, 1024], BF16, tag="xs")
                nc.sync.dma_start(xt[:], x_r[:, i, ssl])
            for j in range(2):
                for ns in range(2):
                    nc.tensor.matmul(
                        ps[j][:, ns * 512:(ns + 1) * 512],
                        w_sb[:, i, j * P:(j + 1) * P],
                        xt[:, ns * 512:(ns + 1) * 512],
                        start=(i == 0), stop=(i == 7),
                    )
        for j in range(2):
            if name == "k":
                nc.vector.tensor_scalar_add(dstTs[j][sb][:, :], ps[j][:],
                                            b_sb[:, j:j + 1])
            else:
                nc.vector.tensor_scalar_add(dstTs[j][:, ssl], ps[j][:],
                                            b_sb[:, j:j + 1])

    nc.sync.dma_start(wk_sb[:], wkT[:].rearrange("(c p) j -> p c j", p=P))
    proj_block("q", xqT_r, wq_sb, bq_sb, QTs, 0)
    nc.sync.dma_start(wv_sb[:], wvT[:].rearrange("(c p) j -> p c j", p=P))
    proj_block("k", xkT_r, wk_sb, bk_sb, KT4, 0)
    proj_block("k", xkT_r, wk_sb, bk_sb, KT4, 1)
    nc.sync.dma_start(wo_sb[:], woT[:].rearrange("(c p) m -> p c m", p=P))
    # V: natural layout, s on partitions
    for g_ss in range(16):
        xvt = xs.tile([P, 8, P], BF16, tag="xs", name=f"xv{g_ss}")
        nc.sync.dma_start(xvt[:], xvT_r[:, :, g_ss * P:(g_ss + 1) * P])
        pv = psB.tile([P, 1024], F32, tag="psB")
        for i in range(8):
            nc.tensor.matmul(
                pv[:, 0:GW], xvt[:, i, :], wv_sb[:, i, :],
                start=(i == 0), stop=(i == 7),
            )
        nc.vector.tensor_tensor(
            Vaugs[g_ss // 8][:, g_ss % 8, :, 0:DK],
            pv[:, 0:GW].rearrange("p (h d) -> p h d", h=HPG),
            bvb[:].rearrange("p (h d) -> p h d", h=HPG),
            ADD,
        )

    # ---- phase 2: attention per (head, q-block) ------------------------
    def attn_qblock(qb, defer=False):
        for h in range(HPG):
            pr = 64 * (h % 2)   # partition offset of this head's features
            jc = h // 2         # feature chunk
            qsl = slice(qb * 1024, (qb + 1) * 1024)
            st = sx.tile([P, 16, 1024], BF16, tag="stexp")
            po = psB.tile([P, 1024], F32, tag="psB")
            for k in range(16):
                pst = psA.tile([P, 1024], F32, tag="psA")
                for ns in range(2):
                    nc.tensor.matmul(
                        pst[:, ns * 512:(ns + 1) * 512],
                        KT4[jc][k // 8][pr:pr + DK, (k % 8) * P:(k % 8 + 1) * P],
                        QTs[jc][pr:pr + DK, qb * 1024 + ns * 512:
                                qb * 1024 + (ns + 1) * 512],
                        start=True, stop=True,
                    )
                nc.scalar.activation(out=st[:, k, :], in_=pst[:], func=EXPF,
                                     scale=0.125)
                for ns in range(2):
                    nc.tensor.matmul(
                        po[0:DK + 1, ns * 512:(ns + 1) * 512],
                        Vaugs[k // 8][:, k % 8, h, :],
                        st[:, k, ns * 512:(ns + 1) * 512],
                        start=(k == 0), stop=(k == 15),
                    )
            # normalize: row DK of po holds softmax denominators
            bc = nrm.tile([DK, 1024], F32, tag="bcast")
            dn = nrm.tile([1, 1024], F32, tag="denom")
            nc.vector.tensor_copy(out=dn[:], in_=po[DK:DK + 1, :])
            nc.vector.reciprocal_approx_fast(bc[0:1, :], dn[:])
            nc.gpsimd.partition_broadcast(bc[:], bc[0:1, :])
            # write O.T for this (head, q-block) into QT's now-dead region
            nc.vector.tensor_tensor(QTs[jc][pr:pr + DK, qsl], po[0:DK, :], bc[:],
                                    MULT)

    def oproj_qblock(qb):
        # output projection for one q-block (overlaps other work)
        for sc in range(qb * 8, (qb + 1) * 8):
            pso = psB.tile([P, 1024], F32, tag="psB")
            for hd in range(2):
                for ms in range(2):
                    nc.tensor.matmul(
                        pso[:, ms * 512:(ms + 1) * 512],
                        QTs[hd][:, sc * P:(sc + 1) * P],
                        wo_sb[:, hd, ms * 512:(ms + 1) * 512],
                        start=(hd == 0), stop=(hd == 1),
                    )
            ot = outp.tile([P, 1024], F32, tag="osb")
            if qb == 1 and sc % 2 == 1:
                nc.scalar.copy(out=ot[:], in_=pso[:])
            else:
                nc.vector.tensor_copy(out=ot[:], in_=pso[:])
            nc.sync.dma_start(out[sc * P:(sc + 1) * P, :], ot[:])

    attn_qblock(0)
    # Q projection for the second s-half, hidden under qb0 attention; O-proj
    # for qb0 comes after it so qb1's first scores aren't serialized behind it
    proj_block("q", xqT_r, wq_sb, bq_sb, QTs, 1)
    oproj_qblock(0)
    attn_qblock(1)
    oproj_qblock(1)



_prog_cache = {}


def _build_program():
    if "nc" not in _prog_cache:
        from contextlib import ExitStack
        nc = bacc.Bacc("TRN2", target_bir_lowering=False)
        with tile.TileContext(nc) as tc:
            with ExitStack() as ctx:
                _emit(nc, tc, ctx)
        nc.compile()
        _prog_cache["nc"] = nc
    return _prog_cache["nc"]


def make_in_maps(query, key, value, Wq, bq, Wk, bk, Wv, bv, Wo, bo):
    query, key, value = (np.asarray(t, np.float32) for t in (query, key, value))
    Wq, Wk, Wv, Wo = (np.asarray(t, np.float32) for t in (Wq, Wk, Wv, Wo))
    bq, bk, bv = (np.asarray(t, np.float32) for t in (bq, bk, bv))
    xT = {b: {} for b in range(B)}
    for b in range(B):
        xT[b]["q"] = np.ascontiguousarray(query[b].T).astype(np.float16)
        xT[b]["k"] = np.ascontiguousarray(key[b].T).astype(np.float16)
        xT[b]["v"] = np.ascontiguousarray(value[b].T).astype(np.float16)
    in_maps = []
    for c in range(N_CORES):
        b, g = divmod(c, GROUPS)
        gs = slice(g * GW, (g + 1) * GW)
        in_maps.append({
            "xqT": xT[b]["q"], "xkT": xT[b]["k"], "xvT": xT[b]["v"],
            "wqT": np.ascontiguousarray(Wq[gs, :].T).astype(np.float16),
            "wkT": np.ascontiguousarray(Wk[gs, :].T).astype(np.float16),
            "wvT": np.ascontiguousarray(Wv[gs, :].T).astype(np.float16),
            "woT": np.ascontiguousarray(Wo[:, gs].T).astype(np.float16),
            "bq2": np.ascontiguousarray(bq[gs].reshape(2, 128).T),
            "bk2": np.ascontiguousarray(bk[gs].reshape(2, 128).T),
            "bvr": np.ascontiguousarray(bv[gs].reshape(1, GW)),
        })
    return in_maps


def run_on_hw(in_maps, trace=False, **kw):
    nc = _build_program()
    return run_bass_kernel_spmd(nc, in_maps, core_ids=list(range(N_CORES)),
                                trace=trace, **kw)


def kernel(query, key, value, Wq, bq, Wk, bk, Wv, bv, Wo, bo):
    in_maps = make_in_maps(query, key, value, Wq, bq, Wk, bk, Wv, bv, Wo, bo)
    res = run_on_hw(in_maps)
    out = np.zeros((B, S, D_MODEL), np.float32)
    for c in range(N_CORES):
        out[c // GROUPS] += res.results[c]["out"]
    out += np.asarray(bo, np.float32)
    return out


if __name__ == "__main__":
    # self-check against a pure-numpy reference
    rng = np.random.default_rng(0)
    sc = 1.0 / np.sqrt(D_MODEL)
    inp = dict(
        query=rng.standard_normal((B, S, D_MODEL), np.float32),
        key=rng.standard_normal((B, S, D_MODEL), np.float32),
        value=rng.standard_normal((B, S, D_MODEL), np.float32),
        Wq=(rng.standard_normal((D_MODEL, D_MODEL)) * sc).astype(np.float32),
        bq=rng.standard_normal(D_MODEL).astype(np.float32) * 0.1,
        Wk=(rng.standard_normal((D_MODEL, D_MODEL)) * sc).astype(np.float32),
        bk=rng.standard_normal(D_MODEL).astype(np.float32) * 0.1,
        Wv=(rng.standard_normal((D_MODEL, D_MODEL)) * sc).astype(np.float32),
        bv=rng.standard_normal(D_MODEL).astype(np.float32) * 0.1,
        Wo=(rng.standard_normal((D_MODEL, D_MODEL)) * sc).astype(np.float32),
        bo=rng.standard_normal(D_MODEL).astype(np.float32) * 0.1,
    )

    def np_ref(query, key, value, Wq, bq, Wk, bk, Wv, bv, Wo, bo):
        q = query.astype(np.float64) @ Wq.T.astype(np.float64) + bq
        k = key.astype(np.float64) @ Wk.T.astype(np.float64) + bk
        v = value.astype(np.float64) @ Wv.T.astype(np.float64) + bv
        q = q.reshape(B, S, NUM_HEADS, DK).transpose(0, 2, 1, 3)
        k = k.reshape(B, S, NUM_HEADS, DK).transpose(0, 2, 1, 3)
        v = v.reshape(B, S, NUM_HEADS, DK).transpose(0, 2, 1, 3)
        sc_ = np.einsum("bhqd,bhkd->bhqk", q, k) / np.sqrt(DK)
        sc_ -= sc_.max(-1, keepdims=True)
        a = np.exp(sc_)
        a /= a.sum(-1, keepdims=True)
        o = np.einsum("bhqk,bhkd->bhqd", a, v)
        o = o.transpose(0, 2, 1, 3).reshape(B, S, D_MODEL)
        return o @ Wo.T.astype(np.float64) + bo

    exp = np_ref(**inp)
    got = kernel(**inp)
    scale = np.abs(exp).max()
    err = np.abs(got - exp)
    print(f"max abs err {err.max():.4e}  rel {err.max() / scale:.4e}  "
          f"mean rel {err.mean() / scale:.4e}")
